# revision 17
# baseline (speedup 1.0000x reference)
"""Trainium2 Bass kernel for nn_AtomicEnsemble (ANI-style per-species MLP over atoms).

Contract: kernel(**inputs) takes the FULL unsharded numpy inputs
(species [2048,32] i32, aev [2048,32,384] f32, W1..W4/b1..b4 per-species MLP
params) and returns the FULL output energies [2048] f32.

Strategy: data-parallel over molecules (256 molecules -> 8192 atoms per core,
8 cores). Per-species MLP evaluated densely (all species over all atoms) in
feature-major layout on-chip; per-atom species selection via one-hot mask;
per-molecule sum on-chip. CELU is computed exactly via the identity
  celu(z, a)/a = relu(y) + min(exp(y), 1) - 1,   y = z/a
with the 1/a, a and -1 constants folded into the weights/biases on the host.

All constants ship in one packed [128, F] DMA (wpack) so compute instructions
depend on a single DMA lane; primer ops at the start absorb that wait so
matmuls never carry more than one sync wait (walrus LDW limit).
"""
import numpy as np

import concourse.bass as bass
import concourse.bacc as bacc
import concourse.tile as tile
import concourse.mybir as mybir
from concourse.bass_utils import run_bass_kernel_spmd

AF = mybir.ActivationFunctionType
OP = mybir.AluOpType
FP32 = mybir.dt.float32
BF16 = mybir.dt.bfloat16

B, A, D = 2048, 32, 384
S = 4
ALPHA = 0.1
NCORES = 8
BC = B // NCORES          # molecules per core
N = BC * A                # atoms per core (8192)
NT = N // 512             # 512-atom tiles per core (16)

LAST_RUN = None           # BassKernelResults of the most recent run (for test harness)
_CACHE = {}

# wpack column offsets (f32 [128, F])
_OFF = {}
_F = 0
for _name, _w in [("ident", 128), ("w1", 3 * 8 * 128), ("b1", 8),
                  ("w2", 8 * 128), ("b2", 4), ("w3", 4 * 96), ("b3", 4),
                  ("w4", 4 * 4), ("b4c", 1), ("ones4", 1)]:
    _OFF[_name] = _F
    _F += _w
WPACK_F = _F


# --------------------------------------------------------------------------
# Host-side weight packing
# --------------------------------------------------------------------------
def pack_weights(W1, b1, W2, b2, W3, b3, W4, b4):
    a = ALPHA
    wp = np.zeros((128, WPACK_F), np.float32)
    wp[:, 0:128] = np.eye(128, dtype=np.float32)

    w1 = wp[:, _OFF["w1"]:_OFF["w1"] + 3 * 8 * 128].reshape(128, 3, 8, 128)
    b1v = wp[:, _OFF["b1"]:_OFF["b1"] + 8]
    for s in range(S):
        for h in range(2):
            width = 128 if h == 0 else 32
            for k in range(3):
                w1[:, k, 2 * s + h, 0:width] = \
                    W1[s, 128 * k:128 * (k + 1), 128 * h:128 * h + width] / a
            b1v[0:width, 2 * s + h] = b1[s, 128 * h:128 * h + width] / a

    w2 = wp[:, _OFF["w2"]:_OFF["w2"] + 8 * 128].reshape(128, 8, 128)
    for s in range(S):
        w2[0:128, 2 * s, :] = W2[s, 0:128, :]
        w2[0:32, 2 * s + 1, :] = W2[s, 128:160, :]
    wp[:, _OFF["b2"]:_OFF["b2"] + 4] = (b2 / a - W2.sum(axis=1)).T

    w3 = wp[:, _OFF["w3"]:_OFF["w3"] + 4 * 96].reshape(128, 4, 96)
    for s in range(S):
        w3[:, s, :] = W3[s]
    wp[0:96, _OFF["b3"]:_OFF["b3"] + 4] = (b3 / a - W3.sum(axis=1)).T

    w4 = wp[:, _OFF["w4"]:_OFF["w4"] + 16].reshape(128, 4, 4)
    for s in range(S):
        w4[0:96, s, s] = a * W4[s, :, 0]
    wp[0:4, _OFF["b4c"]] = b4[:, 0] - a * W4[:, :, 0].sum(axis=1)
    wp[0:4, _OFF["ones4"]] = 1.0
    return wp


def legalize_sync_waits(nc):
    """Walrus allows ~2 sync commands (waits+updates) per instruction; hoist
    excess waits onto same-engine NOPs placed directly before the instruction
    (the engine stream is sequential, so semantics are preserved)."""
    for bb in nc.main_func.blocks:
        out = []
        for inst in bb.instructions:
            si = inst.sync_info
            if si is not None:
                waits, ups = list(si.on_wait), list(si.on_update)
                budget = min(1, max(2 - len(ups), 0))
                if len(waits) > budget:
                    excess = waits[:len(waits) - budget]
                    keep = waits[len(waits) - budget:]
                    for w in excess:
                        nop = mybir.InstNoOp(name=nc.get_next_instruction_name())
                        nop.engine = inst.engine
                        nop.sync_info = mybir.SyncInfo(on_wait=[w], on_update=[])
                        nc.register_instruction(nop)
                        out.append(nop)
                    inst.sync_info = mybir.SyncInfo(on_wait=keep, on_update=ups)
            out.append(inst)
        bb.instructions = out


# --------------------------------------------------------------------------
# Device program (SPMD, one build shared by all 8 cores)
# --------------------------------------------------------------------------
def _celu(nc, act_pool, h_out, pz, bias_ap):
    """h_out = relu(y) + min(exp(y),1), y = pz + bias (the -1 is folded into
    the next layer's bias). pz is PSUM, h_out SBUF."""
    P, Nf = pz.shape[0], pz.shape[-1]
    u = act_pool.tile([128, 512], FP32, tag="u")
    r = act_pool.tile([128, 512], FP32, tag="r")
    v = act_pool.tile([128, 512], FP32, tag="v")
    nc.scalar.activation(u[0:P, 0:Nf], pz, AF.Exp, bias=bias_ap)
    nc.scalar.activation(r[0:P, 0:Nf], pz, AF.Relu, bias=bias_ap)
    nc.vector.tensor_scalar(v[0:P, 0:Nf], u[0:P, 0:Nf], 1.0, None, op0=OP.min)
    nc.vector.tensor_tensor(h_out, v[0:P, 0:Nf], r[0:P, 0:Nf], op=OP.add)


def build_program():
    nc = bacc.Bacc(None, target_bir_lowering=False, debug=False)
    aev_d = nc.declare_dram_parameter("aev_s", [NT // 4, 16, 128, 384], FP32,
                                      isOutput=False)
    mask_d = nc.declare_dram_parameter("mask4", [4, N], BF16, isOutput=False)
    wp_d = nc.declare_dram_parameter("wpack", [128, WPACK_F], FP32, isOutput=False)
    out_d = nc.declare_dram_parameter("energies", [1, BC], FP32, isOutput=True)

    with tile.TileContext(nc) as tc:
        with (
            tc.tile_pool(name="const", bufs=1) as cpool,
            tc.tile_pool(name="xin", bufs=2) as xin_pool,
            tc.tile_pool(name="xt", bufs=2) as xt_pool,
            tc.tile_pool(name="hs", bufs=2) as h_pool,
            tc.tile_pool(name="act", bufs=3) as act_pool,
            tc.tile_pool(name="pT", bufs=2, space="PSUM") as pT_pool,
            tc.tile_pool(name="pz", bufs=3, space="PSUM") as pz_pool,
            tc.tile_pool(name="pe", bufs=1, space="PSUM") as pe_pool,
        ):
            wp = cpool.tile([128, WPACK_F], FP32, tag="wpack")
            nc.sync.dma_start(wp[:], wp_d[:])
            mask = cpool.tile([4, N], BF16, tag="mask")
            nc.sync.dma_start(mask[:], mask_d[:])
            eacc = cpool.tile([4, BC], FP32, tag="eacc")

            def wslice(name, count):
                return wp[:, _OFF[name]:_OFF[name] + count]

            ident = wslice("ident", 128)
            w1 = wslice("w1", 3 * 8 * 128).rearrange("p (k c x) -> p k c x", k=3, c=8)
            b1 = wslice("b1", 8)
            w2 = wslice("w2", 8 * 128).rearrange("p (c x) -> p c x", c=8)
            b2 = wslice("b2", 4)
            w3 = wslice("w3", 4 * 96).rearrange("p (c x) -> p c x", c=4)
            b3 = wslice("b3", 4)
            w4 = wslice("w4", 16).rearrange("p (c x) -> p c x", c=4)
            b4 = wslice("b4c", 1)
            ones4 = wslice("ones4", 1)

            # primers: absorb the wpack/mask DMA waits so loop instructions
            # (especially matmuls: 1-wait LDW limit) never wait on DMA lanes
            prime_ps = pT_pool.tile([128, 128], FP32, tag="pT")
            nc.tensor.transpose(prime_ps[:], ident, ident)
            prime_sb = act_pool.tile([128, 512], FP32, tag="u")
            nc.scalar.copy(prime_sb[0:128, 0:128], prime_ps[:])
            prime_v = act_pool.tile([128, 512], FP32, tag="v")
            nc.vector.tensor_copy(prime_v[0:4, 0:16], mask[:, 0:16])

            for tq in range(NT // 4):
              # one big DMA per 4 atom-tiles (2048 atoms) to stay within the
              # per-instruction sync-wait budget and DMA lane count
              x_am = xin_pool.tile([128, 16, 384], FP32)
              nc.sync.dma_start(x_am[:], aev_d[tq].rearrange("b p d -> p b d"))
              for t4 in range(4):
                t = 4 * tq + t4

                # ---- transpose to feature-major XT [128p, 3k, 512]
                xt = xt_pool.tile([128, 3, 512], FP32)
                for bblk in range(4):
                    for k in range(3):
                        pt = pT_pool.tile([128, 128], FP32, tag="pT")
                        nc.tensor.transpose(
                            pt[:], x_am[:, 4 * t4 + bblk, 128 * k:128 * (k + 1)],
                            ident)
                        nc.vector.tensor_copy(
                            xt[:, k, 128 * bblk:128 * (bblk + 1)], pt[:])

                # ---- L1: per species, output chunks 128 + 32
                h1 = h_pool.tile([128, 8, 512], FP32, tag="h1")
                for s in range(S):
                    for hh in range(2):
                        width = 128 if hh == 0 else 32
                        ci = 2 * s + hh
                        pz = pz_pool.tile([128, 512], FP32, tag="pz")
                        for k in range(3):
                            nc.tensor.matmul(pz[0:width, :], w1[:, k, ci, 0:width],
                                             xt[:, k, :],
                                             start=(k == 0), stop=(k == 2))
                        _celu(nc, act_pool, h1[0:width, ci, :], pz[0:width, :],
                              b1[0:width, ci:ci + 1])

                # ---- L2: per species, K-chunks 128 + 32
                h2 = h_pool.tile([128, 4, 512], FP32, tag="h2")
                for s in range(S):
                    pz = pz_pool.tile([128, 512], FP32, tag="pz")
                    nc.tensor.matmul(pz[:], w2[:, 2 * s, :], h1[:, 2 * s, :],
                                     start=True, stop=False)
                    nc.tensor.matmul(pz[:], w2[0:32, 2 * s + 1, :],
                                     h1[0:32, 2 * s + 1, :],
                                     start=False, stop=True)
                    _celu(nc, act_pool, h2[:, s, :], pz[:], b2[:, s:s + 1])

                # ---- L3: per species [128 -> 96]
                h3 = h_pool.tile([128, 4, 512], FP32, tag="h3")
                for s in range(S):
                    pz = pz_pool.tile([128, 512], FP32, tag="pz")
                    nc.tensor.matmul(pz[0:96, :], w3[:, s, :], h2[:, s, :],
                                     start=True, stop=True)
                    _celu(nc, act_pool, h3[0:96, s, :], pz[0:96, :],
                          b3[0:96, s:s + 1])

                # ---- L4: 4 accumulating matmuls -> e_all [4, 512]
                pe4 = pe_pool.tile([4, 512], FP32, tag="pe4")
                for s in range(S):
                    nc.tensor.matmul(pe4[:], w4[0:96, s, :], h3[0:96, s, :],
                                     start=(s == 0), stop=(s == 3))
                e_sb = act_pool.tile([4, 512], FP32, tag="esb")
                nc.scalar.activation(e_sb[:], pe4[:], AF.Identity, bias=b4[0:4, 0:1])

                # ---- select by species + per-molecule partial sums
                me = act_pool.tile([4, 512], FP32, tag="me")
                nc.vector.tensor_tensor(me[:], e_sb[:],
                                        mask[:, 512 * t:512 * (t + 1)], op=OP.mult)
                nc.vector.tensor_reduce(
                    eacc[:, 16 * t:16 * (t + 1)],
                    me[:].rearrange("p (mm aa) -> p mm aa", aa=32),
                    axis=mybir.AxisListType.X, op=OP.add)

            # ---- final: sum the 4 species rows -> [1, BC] and store
            pf = pe_pool.tile([1, BC], FP32, tag="pf")
            nc.tensor.matmul(pf[:], ones4[0:4, 0:1], eacc[:], start=True, stop=True)
            out_sb = act_pool.tile([1, BC], FP32, tag="out")
            nc.scalar.copy(out_sb[:], pf[:])
            nc.sync.dma_start(out_d[:], out_sb[:])

    nc.compile()
    nc.compile = lambda: None   # guard: finalize() would otherwise recompile
    return nc


# --------------------------------------------------------------------------
# Entry point
# --------------------------------------------------------------------------
def kernel(species, aev, W1, b1, W2, b2, W3, b3, W4, b4):
    global LAST_RUN
    species = np.asarray(species, dtype=np.int32)
    aev = np.ascontiguousarray(np.asarray(aev, dtype=np.float32))

    wpack = pack_weights(np.asarray(W1, np.float32), np.asarray(b1, np.float32),
                         np.asarray(W2, np.float32), np.asarray(b2, np.float32),
                         np.asarray(W3, np.float32), np.asarray(b3, np.float32),
                         np.asarray(W4, np.float32), np.asarray(b4, np.float32))

    if "prog" not in _CACHE:
        _CACHE["prog"] = build_program()
    nc = _CACHE["prog"]

    in_maps = [core_inputs(species, aev, wpack, c) for c in range(NCORES)]

    res = run_bass_kernel_spmd(nc, in_maps, list(range(NCORES)))
    LAST_RUN = res
    out = np.concatenate([res.results[c]["energies"].reshape(BC) for c in range(NCORES)])
    return out.astype(np.float32)

def core_inputs(species, aev, wpack, c):
    import ml_dtypes
    sp = species[c * BC:(c + 1) * BC].reshape(-1)
    aev_s = aev[c * BC:(c + 1) * BC].reshape(N, D)
    mask4 = (sp[None, :] == np.arange(4, dtype=np.int32)[:, None]).astype(ml_dtypes.bfloat16)
    return dict(
        wpack=wpack,
        aev_s=np.ascontiguousarray(aev_s.reshape(NT // 4, 16, 128, 384)),
        mask4=mask4,
    )


def prep_all(inputs):
    """(build nc, per-core in_maps, unshard fn) for the test harness."""
    species = np.asarray(inputs["species"], dtype=np.int32)
    aev = np.ascontiguousarray(np.asarray(inputs["aev"], dtype=np.float32))
    wpack = pack_weights(*[np.asarray(inputs[k], np.float32) for k in
                           ("W1", "b1", "W2", "b2", "W3", "b3", "W4", "b4")])
    nc = build_program()
    in_maps = [core_inputs(species, aev, wpack, c) for c in range(NCORES)]
    unshard = lambda res: np.concatenate(
        [np.asarray(res[c]["energies"]).reshape(BC) for c in range(NCORES)])
    return nc, in_maps, unshard
